# revision 20
# baseline (speedup 1.0000x reference)
"""Trainium2 Bass kernel for DenseRelativeLoc.

Computation (per batch b of 64):
  - gather 256 px-points and 256 py-points (columns of x[b] viewed as
    [C=768, HW=3136]) -> ptsT chunks [128c, 512s] via GPSIMD ap_gather
  - 3-layer MLP on the gathered features via TensorE matmuls in a
    transposed layout (activations kept as [feature-part, sample-free])
  - predxy [B*S, 2] written back; deltaxy computed host-side (pure
    integer arithmetic on the indices)

Sharding: data-parallel over batch, 8 batches per NeuronCore x 8 cores.
MLP weights replicated. No cross-core communication.

Pipeline notes: HWDGE rings effectively allow ~1 outstanding DMA per
issuing engine, and each DMA pays ~7us completion latency before its
consumer semaphore fires. So x is streamed as 16 half-batch loads
(4.7 MB each) alternating between the SP (nc.sync) and ACT (nc.scalar)
HWDGE rings, the 3 chunk-gathers of a half-batch are fused into one
ap_gather, and all constants ride in a single packed preload DMA.
ReLU+bias runs on DVE so the ACT ring stays DMA-only.
"""

import sys
import types
import contextlib
import ctypes

sys.path.insert(0, "/opt/trn_rl_repo")

import numpy as np

# ---------------------------------------------------------------- constants
B, C, H, W = 64, 768, 56, 56
HW = H * W            # 3136
S = 256               # points per batch (per side)
NIDX = 2 * S          # 512 gathered columns per batch (px then py)
NH = 256              # hidden width
OUT = 2
NCORES = 8
NB = B // NCORES      # batches per core = 8
KC = C // 128         # channel chunks = 6
HALF = 3              # chunks per half-batch load
GIDX = HALF * NIDX    # fused gather indices per half-batch = 1536

# packed const tensor column offsets (f32 elements per partition)
O_W1 = 0                       # [128, 12, 256] -> 3072
O_W2 = O_W1 + 2 * KC * NH      # [128, 2, 256] -> 512
O_W3 = O_W2 + 2 * NH           # [128, 2, 2] -> 4
O_B1 = O_W3 + 2 * OUT          # [128, 2]
O_B2 = O_B1 + 2
O_B3 = O_B2 + 2
NCONST = O_B3 + OUT

_PROGRAMS = {}        # cached compiled programs keyed by nb


def _install_ntff_hook():
    """Recreate antenv.axon_hooks (absent in this image) so that
    run_bass_kernel_spmd(trace=True) can register NTFF profiling."""
    import antenv

    if "antenv.axon_hooks" in sys.modules:
        return
    mod = types.ModuleType("antenv.axon_hooks")
    holder = {"hook": None}
    mod.set_axon_ntff_profile_hook = lambda h: holder.__setitem__("hook", h)
    mod.get_axon_ntff_profile_hook = lambda: holder["hook"]
    sys.modules["antenv.axon_hooks"] = mod
    antenv.axon_hooks = mod

    try:
        lib = ctypes.CDLL("/opt/axon/libaxon_pjrt.so")
    except OSError:
        return
    if not hasattr(lib, "axon_start_nrt_profile"):
        return
    lib.axon_start_nrt_profile.argtypes = [ctypes.POINTER(ctypes.c_int64), ctypes.c_size_t]
    lib.axon_start_nrt_profile.restype = ctypes.c_int64
    lib.axon_stop_nrt_profile.argtypes = [ctypes.c_char_p]
    lib.axon_stop_nrt_profile.restype = ctypes.c_int64

    @contextlib.contextmanager
    def _hook(output_dir, device_ids):
        import jax

        jax.devices()
        if device_ids:
            ids = (ctypes.c_int64 * len(device_ids))(*device_ids)
            rc = lib.axon_start_nrt_profile(ids, len(device_ids))
        else:
            rc = lib.axon_start_nrt_profile(None, 0)
        if rc != 0:
            raise RuntimeError(f"axon_start_nrt_profile rc={rc}")
        try:
            yield
        finally:
            n = lib.axon_stop_nrt_profile(str(output_dir).encode())
            print(f"profile: {n} file(s) written to {output_dir}", file=sys.stderr)

    mod.set_axon_ntff_profile_hook(_hook)


def build_program(nb=NB):
    """Build + compile the per-core Bass/Tile program (cached)."""
    if nb in _PROGRAMS:
        return _PROGRAMS[nb]

    import concourse.mybir as mybir
    import concourse.tile as tile
    from concourse import bacc
    from concourse.bass import ts, ds

    f32 = mybir.dt.float32
    i16 = mybir.dt.int16
    ADD = mybir.AluOpType.add
    MAX = mybir.AluOpType.max

    nc = bacc.Bacc("TRN2", target_bir_lowering=False, debug=False, num_devices=NCORES)

    # x is passed partition-major ([128, nb, KC, HW]) so each half-batch
    # load is one 37.6KB-contiguous HBM run per partition -> few, large
    # DMA descriptors (HWDGE descgen is ~100ns/descriptor and otherwise
    # caps effective bandwidth at ~120 GB/s)
    x_d = nc.dram_tensor("x", [128, nb, KC, HW], f32, kind="ExternalInput")
    idx_d = nc.dram_tensor("idx", [128, nb, KC * NIDX // 16], i16, kind="ExternalInput")
    cst_d = nc.dram_tensor("cst", [128, NCONST], f32, kind="ExternalInput")
    pred_d = nc.dram_tensor("pred", [128, nb, 2, OUT], f32, kind="ExternalOutput")

    with tile.TileContext(nc) as tc:
        with (
            tc.tile_pool(name="xp", bufs=2) as xp,
            tc.tile_pool(name="gp", bufs=1) as gp,
            tc.tile_pool(name="wp", bufs=1) as wp,
            tc.tile_pool(name="hp", bufs=2) as hp,
            tc.tile_pool(name="op", bufs=1) as op,
            tc.tile_pool(name="ps1a", bufs=2, space="PSUM") as ps1a,
            tc.tile_pool(name="ps1b", bufs=2, space="PSUM") as ps1b,
            tc.tile_pool(name="ps2", bufs=2, space="PSUM") as ps2,
            tc.tile_pool(name="ps3", bufs=2, space="PSUM") as ps3,
        ):
            cst = wp.tile([128, NCONST], f32, tag="cst")
            idxt = wp.tile([128, nb, KC * NIDX // 16], i16, tag="idx")
            predt = op.tile([128, nb, 2, OUT], f32, tag="pred")

            nc.sync.dma_start(cst[:], cst_d.ap())
            nc.scalar.dma_start(idxt[:], idx_d.ap())

            def w1ap(j, nh):  # lhsT [128c, 128n] for W1 chunk j, n-half nh
                return cst[:, ds(O_W1 + j * NH + nh * 128, 128)]

            def w2ap(nk, mh):
                return cst[:, ds(O_W2 + nk * NH + mh * 128, 128)]

            def w3ap(mk):
                return cst[:, ds(O_W3 + mk * OUT, OUT)]

            for b in range(nb):
                # ---- load the whole batch (6 chunk DMAs into one tile,
                # alternating HWDGE rings), then ONE fused ap_gather for all
                # 6 chunks x 512 columns. ap_gather costs ~14us of GPSIMD
                # dispatch per call regardless of size, so call it once per
                # batch (num_elems=6*3136=18816 is under the 2^15 limit).
                xt = xp.tile([128, KC, HW], f32, tag="x")
                eng = nc.sync if b % 2 == 0 else nc.scalar
                eng.dma_start(xt[:], x_d.ap()[:, b, :, :])
                g = gp.tile([128, KC * NIDX], f32, tag="g")
                nc.gpsimd.ap_gather(
                    g[:], xt[:], idxt[:, b, :],
                    channels=128, num_elems=KC * HW, d=1, num_idxs=KC * NIDX,
                )
                h1pa = ps1a.tile([128, NH], f32, tag="h1pa")
                h1pb = ps1b.tile([128, NH], f32, tag="h1pb")
                h1p = [h1pa, h1pb]
                for nh in range(2):
                    for k in range(KC):
                        for xy in range(2):
                            nc.tensor.matmul(
                                h1p[nh][:],
                                w1ap(xy * KC + k, nh),
                                g[:, ds(k * NIDX + xy * S, S)],
                                start=(k == 0 and xy == 0),
                                stop=(k == KC - 1 and xy == 1),
                            )
                h1 = hp.tile([128, 2, NH], f32, tag="h1")
                for nh in range(2):
                    nc.vector.tensor_scalar(
                        h1[:, nh, :], h1p[nh][:],
                        cst[:, ds(O_B1 + nh, 1)], 0.0, op0=ADD, op1=MAX,
                    )

                # ---- layer 2
                h2p = ps2.tile([128, 2, NH], f32, tag="h2p")
                for mh in range(2):
                    for nk in range(2):
                        nc.tensor.matmul(
                            h2p[:, mh, :],
                            w2ap(nk, mh),
                            h1[:, nk, :],
                            start=(nk == 0),
                            stop=(nk == 1),
                        )
                h2 = hp.tile([128, 2, NH], f32, tag="h2")
                for mh in range(2):
                    nc.vector.tensor_scalar(
                        h2[:, mh, :], h2p[:, mh, :],
                        cst[:, ds(O_B2 + mh, 1)], 0.0, op0=ADD, op1=MAX,
                    )

                # ---- layer 3 + b3
                pp = ps3.tile([128, 2, OUT], f32, tag="pp")
                for sh in range(2):
                    for mk in range(2):
                        nc.tensor.matmul(
                            pp[:, sh, :],
                            h2[:, mk, ts(sh, 128)],
                            w3ap(mk),
                            start=(mk == 0),
                            stop=(mk == 1),
                        )
                for sh in range(2):
                    nc.vector.tensor_scalar(
                        predt[:, b, sh, :], pp[:, sh, :],
                        cst[:, ds(O_B3, 1)], None, op0=ADD,
                    )

            nc.sync.dma_start(pred_d.ap(), predt[:])

    nc.compile()
    from concourse.bass_interp import get_hw_module

    nc.m = get_hw_module(nc.m)
    _PROGRAMS[nb] = nc
    return nc


def _prep_core_inputs(x, pxs, pys, W1, b1, W2, b2, W3, b3):
    """Host-side shard + layout massage. Returns list of 8 in_maps."""
    x = np.asarray(x, dtype=np.float32).reshape(B, C, HW)
    pxs = np.asarray(pxs).astype(np.int64)
    pys = np.asarray(pys).astype(np.int64)

    # fused gather indices over all 6 chunks: [B, 3072] int16 (chunk k's
    # column j at k*HW + idx), wrapped [16, 192] and replicated to 128
    # partitions (each GPSIMD core reads its own 16)
    xi = pxs[:, :, 0] * H + pxs[:, :, 1]          # [B, S]
    yi = pys[:, :, 0] * H + pys[:, :, 1]          # [B, S]
    base = np.concatenate([xi, yi], axis=1)       # [B, 512]
    fused = np.concatenate([base + k * HW for k in range(KC)], axis=1).astype(np.int16)
    NW = KC * NIDX // 16
    wrapped = fused.reshape(B, NW, 16).transpose(0, 2, 1)    # [B, 16, NW]
    idx128 = np.tile(wrapped, (1, 8, 1)).reshape(B, 128, NW)

    w1k = np.asarray(W1, dtype=np.float32).reshape(2 * KC, 128, NH).transpose(1, 0, 2)
    w2k = np.asarray(W2, dtype=np.float32).reshape(2, 128, NH).transpose(1, 0, 2)
    w3k = np.asarray(W3, dtype=np.float32).reshape(2, 128, OUT).transpose(1, 0, 2)
    b1t = np.asarray(b1, dtype=np.float32).reshape(2, 128).T
    b2t = np.asarray(b2, dtype=np.float32).reshape(2, 128).T
    b3t = np.broadcast_to(np.asarray(b3, dtype=np.float32), (128, OUT))

    cst = np.concatenate(
        [
            w1k.reshape(128, -1),
            w2k.reshape(128, -1),
            w3k.reshape(128, -1),
            b1t,
            b2t,
            b3t,
        ],
        axis=1,
    ).astype(np.float32)
    cst = np.ascontiguousarray(cst)
    assert cst.shape == (128, NCONST)

    in_maps = []
    for c in range(NCORES):
        sl = slice(c * NB, (c + 1) * NB)
        nb = sl.stop - sl.start
        xh = np.ascontiguousarray(
            x[sl].reshape(nb, KC, 128, HW).transpose(2, 0, 1, 3)
        )  # [128, nb, KC, HW], partition-major
        in_maps.append(
            {
                "x": xh,
                "idx": np.ascontiguousarray(idx128[sl].transpose(1, 0, 2)),
                "cst": cst,
            }
        )
    return in_maps


def _assemble_pred(results):
    """Per-core pred [128, NB, 2, 2] -> full predxy [B*S, 2]."""
    parts = []
    for c in range(NCORES):
        p = results[c]["pred"]  # [128, NB, 2, OUT]
        parts.append(np.ascontiguousarray(p.transpose(1, 2, 0, 3)).reshape(NB * 2 * 128, OUT))
    return np.concatenate(parts, axis=0)


def _run(inputs, trace=False):
    _install_ntff_hook()
    from concourse import bass_utils

    nc = build_program()
    in_maps = _prep_core_inputs(**inputs)
    res = bass_utils.run_bass_kernel_spmd(
        nc, in_maps, core_ids=list(range(NCORES)), trace=trace
    )
    predxy = _assemble_pred(res.results)

    pxs = np.asarray(inputs["pxs"]).astype(np.int64)
    pys = np.asarray(inputs["pys"]).astype(np.int64)
    deltaxy = (pxs - pys).astype(np.float32).reshape(-1, 2) + np.float32(H - 1)
    return (predxy, deltaxy), res


def kernel(**inputs):
    outs, _ = _run(inputs, trace=False)
    return outs


# revision 22
# speedup vs baseline: 2.1247x; 2.1247x over previous
"""Trainium2 Bass kernel for DenseRelativeLoc.

Strategy (data-parallel over batch, 8 batches/core x 8 cores):
  - x is host-repacked into a bf16 hi/lo-split gather table
    [128, nb, HW, 12]: partition p, position pos, word j=2*ck+t holds
    bf16 chunk ck (channel ck*128+p) of x (t=0) or of the bf16
    residual (t=1). Streamed to SBUF one batch per DMA (~38KB/part).
  - ONE GPSIMD ap_gather per batch with d=12 fetches, for each of the
    512 sample positions, all 6 channel-chunks x (hi,lo) in a single
    index (ap_gather cost is ~28ns/index independent of d, so fusing
    the chunk dimension into d is ~6x cheaper than per-chunk gathers).
  - 3-layer MLP on TensorE: layer 1 in bf16 with 3-term compensation
    (XhiWhi + XhiWlo + XloWhi ~ fp32 accuracy), layers 2/3 in fp32.
    Activations stay [feature-part, sample-free]; ReLU+bias on DVE.
  - deltaxy is exact integer arithmetic, computed on host.
"""

import sys
import types
import contextlib
import ctypes

sys.path.insert(0, "/opt/trn_rl_repo")

import numpy as np

# ---------------------------------------------------------------- constants
B, C, H, W = 64, 768, 56, 56
HW = H * W            # 3136
S = 256               # points per batch (per side)
NIDX = 2 * S          # 512 gathered positions per batch (px then py)
NH = 256              # hidden width
OUT = 2
NCORES = 8
NB = B // NCORES      # batches per core = 8
KC = C // 128         # channel chunks = 6
D = 2 * KC            # gather payload words per index (6 chunks x hi|lo)

# packed f32 const tensor column offsets (per partition)
O_W2 = 0                       # [128, 2, 256] -> 512
O_W3 = O_W2 + 2 * NH           # [128, 2, 2] -> 4
O_B1 = O_W3 + 2 * OUT          # [128, 2]
O_B2 = O_B1 + 2
O_B3 = O_B2 + 2
NCONST = O_B3 + OUT

_PROGRAMS = {}


def _install_ntff_hook():
    """Recreate antenv.axon_hooks (absent in this image) so that
    run_bass_kernel_spmd(trace=True) can register NTFF profiling."""
    import antenv

    if "antenv.axon_hooks" in sys.modules:
        return
    mod = types.ModuleType("antenv.axon_hooks")
    holder = {"hook": None}
    mod.set_axon_ntff_profile_hook = lambda h: holder.__setitem__("hook", h)
    mod.get_axon_ntff_profile_hook = lambda: holder["hook"]
    sys.modules["antenv.axon_hooks"] = mod
    antenv.axon_hooks = mod

    try:
        lib = ctypes.CDLL("/opt/axon/libaxon_pjrt.so")
    except OSError:
        return
    if not hasattr(lib, "axon_start_nrt_profile"):
        return
    lib.axon_start_nrt_profile.argtypes = [ctypes.POINTER(ctypes.c_int64), ctypes.c_size_t]
    lib.axon_start_nrt_profile.restype = ctypes.c_int64
    lib.axon_stop_nrt_profile.argtypes = [ctypes.c_char_p]
    lib.axon_stop_nrt_profile.restype = ctypes.c_int64

    @contextlib.contextmanager
    def _hook(output_dir, device_ids):
        import jax

        jax.devices()
        if device_ids:
            ids = (ctypes.c_int64 * len(device_ids))(*device_ids)
            rc = lib.axon_start_nrt_profile(ids, len(device_ids))
        else:
            rc = lib.axon_start_nrt_profile(None, 0)
        if rc != 0:
            raise RuntimeError(f"axon_start_nrt_profile rc={rc}")
        try:
            yield
        finally:
            n = lib.axon_stop_nrt_profile(str(output_dir).encode())
            print(f"profile: {n} file(s) written to {output_dir}", file=sys.stderr)

    mod.set_axon_ntff_profile_hook(_hook)


def build_program(nb=NB):
    """Build + compile the per-core Bass/Tile program (cached)."""
    if nb in _PROGRAMS:
        return _PROGRAMS[nb]

    import concourse.mybir as mybir
    import concourse.tile as tile
    from concourse import bacc
    from concourse.bass import ts, ds

    f32 = mybir.dt.float32
    bf16 = mybir.dt.bfloat16
    i16 = mybir.dt.int16
    ADD = mybir.AluOpType.add
    MAX = mybir.AluOpType.max

    nc = bacc.Bacc("TRN2", target_bir_lowering=False, debug=False, num_devices=NCORES)

    x_d = nc.dram_tensor("x", [128, nb, HW, D], bf16, kind="ExternalInput")
    idx_d = nc.dram_tensor("idx", [128, nb, NIDX // 16], i16, kind="ExternalInput")
    wb_d = nc.dram_tensor("wb", [128, 2 * 2 * KC, NH], bf16, kind="ExternalInput")
    cst_d = nc.dram_tensor("cst", [128, NCONST], f32, kind="ExternalInput")
    pred_d = nc.dram_tensor("pred", [128, nb, 2, OUT], f32, kind="ExternalOutput")

    with tile.TileContext(nc) as tc:
        with (
            tc.tile_pool(name="xp", bufs=2) as xp,
            tc.tile_pool(name="gp", bufs=2) as gp,
            tc.tile_pool(name="wp", bufs=1) as wp,
            tc.tile_pool(name="hp", bufs=2) as hp,
            tc.tile_pool(name="op", bufs=1) as op,
            tc.tile_pool(name="ps1a", bufs=2, space="PSUM") as ps1a,
            tc.tile_pool(name="ps1b", bufs=2, space="PSUM") as ps1b,
            tc.tile_pool(name="ps2", bufs=2, space="PSUM") as ps2,
            tc.tile_pool(name="ps3", bufs=2, space="PSUM") as ps3,
        ):
            cst = wp.tile([128, NCONST], f32, tag="cst")
            wb = wp.tile([128, 2 * 2 * KC, NH], bf16, tag="wb")
            idxt = wp.tile([128, nb, NIDX // 16], i16, tag="idx")
            predt = op.tile([128, nb, 2, OUT], f32, tag="pred")

            nc.sync.dma_start(cst[:], cst_d.ap())
            nc.sync.dma_start(wb[:], wb_d.ap())
            nc.scalar.dma_start(idxt[:], idx_d.ap())

            def w1ap(t, j, nh):  # lhsT [128c, 128n]: t=0 hi, 1 lo; j=xy*KC+ck
                return wb[:, t * 2 * KC + j, ts(nh, 128)]

            def w2ap(nk, mh):
                return cst[:, ds(O_W2 + nk * NH + mh * 128, 128)]

            def w3ap(mk):
                return cst[:, ds(O_W3 + mk * OUT, OUT)]

            for b in range(nb):
                xt = xp.tile([128, HW, D], bf16, tag="x")
                eng = nc.sync if b % 2 == 0 else nc.scalar
                eng.dma_start(xt[:], x_d.ap()[:, b, :, :])
                g = gp.tile([128, NIDX, D], bf16, tag="g")
                nc.gpsimd.ap_gather(
                    g[:], xt[:], idxt[:, b, :],
                    channels=128, num_elems=HW, d=D, num_idxs=NIDX,
                )

                # ---- layer 1: 3-term bf16 compensation.
                # g[:, i, 2*ck+t] = chunk ck (hi t=0 / lo t=1) of sample i.
                h1pa = ps1a.tile([128, NH], f32, tag="h1pa")
                h1pb = ps1b.tile([128, NH], f32, tag="h1pb")
                h1p = [h1pa, h1pb]
                for nh in range(2):
                    first = True
                    for ck in range(KC):
                        for xy in range(2):
                            rhs_hi = g[:, ds(xy * S, S), 2 * ck]
                            rhs_lo = g[:, ds(xy * S, S), 2 * ck + 1]
                            terms = [
                                (w1ap(0, xy * KC + ck, nh), rhs_hi),
                                (w1ap(1, xy * KC + ck, nh), rhs_hi),
                                (w1ap(0, xy * KC + ck, nh), rhs_lo),
                            ]
                            for ti, (wa, ra) in enumerate(terms):
                                last = ck == KC - 1 and xy == 1 and ti == 2
                                nc.tensor.matmul(
                                    h1p[nh][:], wa, ra,
                                    start=first, stop=last,
                                )
                                first = False
                h1 = hp.tile([128, 2, NH], f32, tag="h1")
                for nh in range(2):
                    nc.vector.tensor_scalar(
                        h1[:, nh, :], h1p[nh][:],
                        cst[:, ds(O_B1 + nh, 1)], 0.0, op0=ADD, op1=MAX,
                    )

                # ---- layer 2 (fp32)
                h2p = ps2.tile([128, 2, NH], f32, tag="h2p")
                for mh in range(2):
                    for nk in range(2):
                        nc.tensor.matmul(
                            h2p[:, mh, :],
                            w2ap(nk, mh),
                            h1[:, nk, :],
                            start=(nk == 0),
                            stop=(nk == 1),
                        )
                h2 = hp.tile([128, 2, NH], f32, tag="h2")
                for mh in range(2):
                    nc.vector.tensor_scalar(
                        h2[:, mh, :], h2p[:, mh, :],
                        cst[:, ds(O_B2 + mh, 1)], 0.0, op0=ADD, op1=MAX,
                    )

                # ---- layer 3 (fp32) + b3
                pp = ps3.tile([128, 2, OUT], f32, tag="pp")
                for sh in range(2):
                    for mk in range(2):
                        nc.tensor.matmul(
                            pp[:, sh, :],
                            h2[:, mk, ts(sh, 128)],
                            w3ap(mk),
                            start=(mk == 0),
                            stop=(mk == 1),
                        )
                for sh in range(2):
                    nc.vector.tensor_scalar(
                        predt[:, b, sh, :], pp[:, sh, :],
                        cst[:, ds(O_B3, 1)], None, op0=ADD,
                    )

            nc.sync.dma_start(pred_d.ap(), predt[:])

    nc.compile()
    from concourse.bass_interp import get_hw_module

    nc.m = get_hw_module(nc.m)
    _PROGRAMS[nb] = nc
    return nc


def _prep_core_inputs(x, pxs, pys, W1, b1, W2, b2, W3, b3, nb=NB):
    """Host-side shard + layout massage. Returns list of in_maps."""
    import ml_dtypes

    bf16 = ml_dtypes.bfloat16
    x = np.asarray(x, dtype=np.float32).reshape(B, C, HW)
    pxs = np.asarray(pxs).astype(np.int64)
    pys = np.asarray(pys).astype(np.int64)

    # gather table [B, 128, HW, 12]: word j=2*ck+t at (p, pos) = bf16
    # hi/lo of x[b, ck*128+p, pos]
    xt = np.ascontiguousarray(x.transpose(0, 2, 1))       # [B, HW, C] f32
    xhi = xt.astype(bf16)
    xlo = (xt - xhi.astype(np.float32)).astype(bf16)
    # [B, HW, KC, 128] -> [B, 128, HW, KC]
    hi4 = xhi.reshape(B, HW, KC, 128).transpose(0, 3, 1, 2)
    lo4 = xlo.reshape(B, HW, KC, 128).transpose(0, 3, 1, 2)
    xtab = np.stack([hi4, lo4], axis=4).reshape(B, 128, HW, D)  # j = 2*ck+t

    # flat position indices [B, 512] int16, wrapped [16, 32], tiled to 128
    xi = pxs[:, :, 0] * H + pxs[:, :, 1]
    yi = pys[:, :, 0] * H + pys[:, :, 1]
    base = np.concatenate([xi, yi], axis=1).astype(np.int16)        # [B, 512]
    wrapped = base.reshape(B, NIDX // 16, 16).transpose(0, 2, 1)    # [B, 16, 32]
    idx128 = np.tile(wrapped, (1, 8, 1)).reshape(B, 128, NIDX // 16)

    # weights: hi/lo bf16 W1 chunks as lhsT [128c, j, n]
    W1 = np.asarray(W1, dtype=np.float32)
    w1hi = W1.astype(bf16)
    w1lo = (W1 - w1hi.astype(np.float32)).astype(bf16)
    wk = [m.reshape(2 * KC, 128, NH).transpose(1, 0, 2) for m in (w1hi, w1lo)]
    wb = np.ascontiguousarray(np.concatenate(wk, axis=1))           # [128, 24, 256]

    w2k = np.asarray(W2, dtype=np.float32).reshape(2, 128, NH).transpose(1, 0, 2)
    w3k = np.asarray(W3, dtype=np.float32).reshape(2, 128, OUT).transpose(1, 0, 2)
    b1t = np.asarray(b1, dtype=np.float32).reshape(2, 128).T
    b2t = np.asarray(b2, dtype=np.float32).reshape(2, 128).T
    b3t = np.broadcast_to(np.asarray(b3, dtype=np.float32), (128, OUT))
    cst = np.ascontiguousarray(
        np.concatenate(
            [w2k.reshape(128, -1), w3k.reshape(128, -1), b1t, b2t, b3t], axis=1
        ).astype(np.float32)
    )
    assert cst.shape == (128, NCONST)

    in_maps = []
    for c in range(NCORES):
        sl = slice(c * NB, c * NB + nb)
        in_maps.append(
            {
                "x": np.ascontiguousarray(xtab[sl].transpose(1, 0, 2, 3)),
                "idx": np.ascontiguousarray(idx128[sl].transpose(1, 0, 2)),
                "wb": wb,
                "cst": cst,
            }
        )
    return in_maps


def _assemble_pred(results):
    """Per-core pred [128, NB, 2, 2] -> full predxy [B*S, 2]."""
    parts = []
    for c in range(NCORES):
        p = results[c]["pred"]  # [128, NB, 2, OUT]
        parts.append(np.ascontiguousarray(p.transpose(1, 2, 0, 3)).reshape(NB * 2 * 128, OUT))
    return np.concatenate(parts, axis=0)


def _run(inputs, trace=False):
    _install_ntff_hook()
    from concourse import bass_utils

    nc = build_program()
    in_maps = _prep_core_inputs(**inputs)
    res = bass_utils.run_bass_kernel_spmd(
        nc, in_maps, core_ids=list(range(NCORES)), trace=trace
    )
    predxy = _assemble_pred(res.results)

    pxs = np.asarray(inputs["pxs"]).astype(np.int64)
    pys = np.asarray(inputs["pys"]).astype(np.int64)
    deltaxy = (pxs - pys).astype(np.float32).reshape(-1, 2) + np.float32(H - 1)
    return (predxy, deltaxy), res


def kernel(**inputs):
    outs, _ = _run(inputs, trace=False)
    return outs


# revision 25
# speedup vs baseline: 3.6508x; 1.7183x over previous
"""Trainium2 Bass kernel for DenseRelativeLoc.

Strategy (data-parallel over batch, 8 batches/core x 8 cores):
  - x is host-repacked into a bf16 hi/lo-split gather table
    [128, nb, HW, 12]: partition p, position pos, word j=2*ck+t holds
    bf16 chunk ck (channel ck*128+p) of x (t=0) or of the bf16
    residual (t=1). Streamed to SBUF one batch per DMA (~38KB/part).
  - ONE GPSIMD ap_gather per batch with d=12 fetches, for each of the
    512 sample positions, all 6 channel-chunks x (hi,lo) in a single
    index (ap_gather cost is ~28ns/index independent of d, so fusing
    the chunk dimension into d is ~6x cheaper than per-chunk gathers).
  - 3-layer MLP on TensorE: layer 1 in bf16 with 3-term compensation
    (XhiWhi + XhiWlo + XloWhi ~ fp32 accuracy), layers 2/3 in fp32.
    Activations stay [feature-part, sample-free]; ReLU+bias on DVE.
  - deltaxy is exact integer arithmetic, computed on host.
"""

import sys
import types
import contextlib
import ctypes

sys.path.insert(0, "/opt/trn_rl_repo")

import numpy as np

# ---------------------------------------------------------------- constants
B, C, H, W = 64, 768, 56, 56
HW = H * W            # 3136
S = 256               # points per batch (per side)
NIDX = 2 * S          # 512 gathered positions per batch (px then py)
NH = 256              # hidden width
OUT = 2
NCORES = 8
NB = B // NCORES      # batches per core = 8
KC = C // 128         # channel chunks = 6
D = KC                # gather payload words per index (6 bf16-hi chunks)

# packed f32 const tensor column offsets (per partition)
O_W2 = 0                       # [128, 2, 256] -> 512
O_W3 = O_W2 + 2 * NH           # [128, 2, 2] -> 4
O_B1 = O_W3 + 2 * OUT          # [128, 2]
O_B2 = O_B1 + 2
O_B3 = O_B2 + 2
NCONST = O_B3 + OUT

_PROGRAMS = {}


def _install_ntff_hook():
    """Recreate antenv.axon_hooks (absent in this image) so that
    run_bass_kernel_spmd(trace=True) can register NTFF profiling."""
    import antenv

    if "antenv.axon_hooks" in sys.modules:
        return
    mod = types.ModuleType("antenv.axon_hooks")
    holder = {"hook": None}
    mod.set_axon_ntff_profile_hook = lambda h: holder.__setitem__("hook", h)
    mod.get_axon_ntff_profile_hook = lambda: holder["hook"]
    sys.modules["antenv.axon_hooks"] = mod
    antenv.axon_hooks = mod

    try:
        lib = ctypes.CDLL("/opt/axon/libaxon_pjrt.so")
    except OSError:
        return
    if not hasattr(lib, "axon_start_nrt_profile"):
        return
    lib.axon_start_nrt_profile.argtypes = [ctypes.POINTER(ctypes.c_int64), ctypes.c_size_t]
    lib.axon_start_nrt_profile.restype = ctypes.c_int64
    lib.axon_stop_nrt_profile.argtypes = [ctypes.c_char_p]
    lib.axon_stop_nrt_profile.restype = ctypes.c_int64

    @contextlib.contextmanager
    def _hook(output_dir, device_ids):
        import jax

        jax.devices()
        if device_ids:
            ids = (ctypes.c_int64 * len(device_ids))(*device_ids)
            rc = lib.axon_start_nrt_profile(ids, len(device_ids))
        else:
            rc = lib.axon_start_nrt_profile(None, 0)
        if rc != 0:
            raise RuntimeError(f"axon_start_nrt_profile rc={rc}")
        try:
            yield
        finally:
            n = lib.axon_stop_nrt_profile(str(output_dir).encode())
            print(f"profile: {n} file(s) written to {output_dir}", file=sys.stderr)

    mod.set_axon_ntff_profile_hook(_hook)


def build_program(nb=NB):
    """Build + compile the per-core Bass/Tile program (cached)."""
    if nb in _PROGRAMS:
        return _PROGRAMS[nb]

    import concourse.mybir as mybir
    import concourse.tile as tile
    from concourse import bacc
    from concourse.bass import ts, ds

    f32 = mybir.dt.float32
    bf16 = mybir.dt.bfloat16
    i16 = mybir.dt.int16
    ADD = mybir.AluOpType.add
    MAX = mybir.AluOpType.max

    nc = bacc.Bacc("TRN2", target_bir_lowering=False, debug=False, num_devices=NCORES)

    x_d = nc.dram_tensor("x", [128, nb, HW, D], bf16, kind="ExternalInput")
    idx_d = nc.dram_tensor("idx", [128, nb, NIDX // 16], i16, kind="ExternalInput")
    wb_d = nc.dram_tensor("wb", [128, 2 * 2 * KC, NH], bf16, kind="ExternalInput")
    cst_d = nc.dram_tensor("cst", [128, NCONST], f32, kind="ExternalInput")
    pred_d = nc.dram_tensor("pred", [128, nb, 2, OUT], f32, kind="ExternalOutput")

    with tile.TileContext(nc) as tc:
        with (
            tc.tile_pool(name="xp", bufs=2) as xp,
            tc.tile_pool(name="gp", bufs=2) as gp,
            tc.tile_pool(name="wp", bufs=1) as wp,
            tc.tile_pool(name="hp", bufs=2) as hp,
            tc.tile_pool(name="op", bufs=1) as op,
            tc.tile_pool(name="ps1a", bufs=2, space="PSUM") as ps1a,
            tc.tile_pool(name="ps1b", bufs=2, space="PSUM") as ps1b,
            tc.tile_pool(name="ps2", bufs=2, space="PSUM") as ps2,
            tc.tile_pool(name="ps3", bufs=2, space="PSUM") as ps3,
        ):
            cst = wp.tile([128, NCONST], f32, tag="cst")
            wb = wp.tile([128, 2 * 2 * KC, NH], bf16, tag="wb")
            idxt = wp.tile([128, nb, NIDX // 16], i16, tag="idx")
            predt = op.tile([128, nb, 2, OUT], f32, tag="pred")

            nc.sync.dma_start(cst[:], cst_d.ap())
            nc.sync.dma_start(wb[:], wb_d.ap())
            nc.scalar.dma_start(idxt[:], idx_d.ap())

            def w1ap(t, j, nh):  # lhsT [128c, 128n]: t=0 hi, 1 lo; j=xy*KC+ck
                return wb[:, t * 2 * KC + j, ts(nh, 128)]

            def w2ap(nk, mh):
                return cst[:, ds(O_W2 + nk * NH + mh * 128, 128)]

            def w3ap(mk):
                return cst[:, ds(O_W3 + mk * OUT, OUT)]

            for b in range(nb):
                xt = xp.tile([128, HW, D], bf16, tag="x")
                eng = nc.sync if b % 2 == 0 else nc.scalar
                eng.dma_start(xt[:], x_d.ap()[:, b, :, :])
                g = gp.tile([128, NIDX, D], bf16, tag="g")
                nc.gpsimd.ap_gather(
                    g[:], xt[:], idxt[:, b, :],
                    channels=128, num_elems=HW, d=D, num_idxs=NIDX,
                )

                # ---- layer 1: 2-term bf16 compensation (Xhi(Whi+Wlo));
                # g[:, i, ck] = bf16-hi chunk ck of sample i.
                h1pa = ps1a.tile([128, NH], f32, tag="h1pa")
                h1pb = ps1b.tile([128, NH], f32, tag="h1pb")
                h1p = [h1pa, h1pb]
                for nh in range(2):
                    first = True
                    for ck in range(KC):
                        for xy in range(2):
                            rhs_hi = g[:, ds(xy * S, S), ck]
                            terms = [
                                (w1ap(0, xy * KC + ck, nh), rhs_hi),
                                (w1ap(1, xy * KC + ck, nh), rhs_hi),
                            ]
                            for ti, (wa, ra) in enumerate(terms):
                                last = ck == KC - 1 and xy == 1 and ti == 1
                                nc.tensor.matmul(
                                    h1p[nh][:], wa, ra,
                                    start=first, stop=last,
                                )
                                first = False
                h1 = hp.tile([128, 2, NH], f32, tag="h1")
                for nh in range(2):
                    nc.vector.tensor_scalar(
                        h1[:, nh, :], h1p[nh][:],
                        cst[:, ds(O_B1 + nh, 1)], 0.0, op0=ADD, op1=MAX,
                    )

                # ---- layer 2 (fp32)
                h2p = ps2.tile([128, 2, NH], f32, tag="h2p")
                for mh in range(2):
                    for nk in range(2):
                        nc.tensor.matmul(
                            h2p[:, mh, :],
                            w2ap(nk, mh),
                            h1[:, nk, :],
                            start=(nk == 0),
                            stop=(nk == 1),
                        )
                h2 = hp.tile([128, 2, NH], f32, tag="h2")
                for mh in range(2):
                    nc.vector.tensor_scalar(
                        h2[:, mh, :], h2p[:, mh, :],
                        cst[:, ds(O_B2 + mh, 1)], 0.0, op0=ADD, op1=MAX,
                    )

                # ---- layer 3 (fp32) + b3
                pp = ps3.tile([128, 2, OUT], f32, tag="pp")
                for sh in range(2):
                    for mk in range(2):
                        nc.tensor.matmul(
                            pp[:, sh, :],
                            h2[:, mk, ts(sh, 128)],
                            w3ap(mk),
                            start=(mk == 0),
                            stop=(mk == 1),
                        )
                for sh in range(2):
                    nc.vector.tensor_scalar(
                        predt[:, b, sh, :], pp[:, sh, :],
                        cst[:, ds(O_B3, 1)], None, op0=ADD,
                    )

            nc.sync.dma_start(pred_d.ap(), predt[:])

    nc.compile()
    from concourse.bass_interp import get_hw_module

    nc.m = get_hw_module(nc.m)
    _PROGRAMS[nb] = nc
    return nc


def _prep_core_inputs(x, pxs, pys, W1, b1, W2, b2, W3, b3, nb=NB):
    """Host-side shard + layout massage. Returns list of in_maps."""
    import ml_dtypes

    bf16 = ml_dtypes.bfloat16
    x = np.asarray(x, dtype=np.float32).reshape(B, C, HW)
    pxs = np.asarray(pxs).astype(np.int64)
    pys = np.asarray(pys).astype(np.int64)

    # gather table [B, 128, HW, 6]: word ck at (p, pos) = bf16 hi of
    # x[b, ck*128+p, pos]
    xt = np.ascontiguousarray(x.transpose(0, 2, 1))       # [B, HW, C] f32
    xhi = xt.astype(bf16)
    # [B, HW, KC, 128] -> [B, 128, HW, KC]
    xtab = np.ascontiguousarray(xhi.reshape(B, HW, KC, 128).transpose(0, 3, 1, 2))

    # flat position indices [B, 512] int16, wrapped [16, 32], tiled to 128
    xi = pxs[:, :, 0] * H + pxs[:, :, 1]
    yi = pys[:, :, 0] * H + pys[:, :, 1]
    base = np.concatenate([xi, yi], axis=1).astype(np.int16)        # [B, 512]
    wrapped = base.reshape(B, NIDX // 16, 16).transpose(0, 2, 1)    # [B, 16, 32]
    idx128 = np.tile(wrapped, (1, 8, 1)).reshape(B, 128, NIDX // 16)

    # weights: hi/lo bf16 W1 chunks as lhsT [128c, j, n]
    W1 = np.asarray(W1, dtype=np.float32)
    w1hi = W1.astype(bf16)
    w1lo = (W1 - w1hi.astype(np.float32)).astype(bf16)
    wk = [m.reshape(2 * KC, 128, NH).transpose(1, 0, 2) for m in (w1hi, w1lo)]
    wb = np.ascontiguousarray(np.concatenate(wk, axis=1))           # [128, 24, 256]

    w2k = np.asarray(W2, dtype=np.float32).reshape(2, 128, NH).transpose(1, 0, 2)
    w3k = np.asarray(W3, dtype=np.float32).reshape(2, 128, OUT).transpose(1, 0, 2)
    b1t = np.asarray(b1, dtype=np.float32).reshape(2, 128).T
    b2t = np.asarray(b2, dtype=np.float32).reshape(2, 128).T
    b3t = np.broadcast_to(np.asarray(b3, dtype=np.float32), (128, OUT))
    cst = np.ascontiguousarray(
        np.concatenate(
            [w2k.reshape(128, -1), w3k.reshape(128, -1), b1t, b2t, b3t], axis=1
        ).astype(np.float32)
    )
    assert cst.shape == (128, NCONST)

    in_maps = []
    for c in range(NCORES):
        sl = slice(c * NB, c * NB + nb)
        in_maps.append(
            {
                "x": np.ascontiguousarray(xtab[sl].transpose(1, 0, 2, 3)),
                "idx": np.ascontiguousarray(idx128[sl].transpose(1, 0, 2)),
                "wb": wb,
                "cst": cst,
            }
        )
    return in_maps


def _assemble_pred(results):
    """Per-core pred [128, NB, 2, 2] -> full predxy [B*S, 2]."""
    parts = []
    for c in range(NCORES):
        p = results[c]["pred"]  # [128, NB, 2, OUT]
        parts.append(np.ascontiguousarray(p.transpose(1, 2, 0, 3)).reshape(NB * 2 * 128, OUT))
    return np.concatenate(parts, axis=0)


def _run(inputs, trace=False):
    _install_ntff_hook()
    from concourse import bass_utils

    nc = build_program()
    in_maps = _prep_core_inputs(**inputs)
    res = bass_utils.run_bass_kernel_spmd(
        nc, in_maps, core_ids=list(range(NCORES)), trace=trace
    )
    predxy = _assemble_pred(res.results)

    pxs = np.asarray(inputs["pxs"]).astype(np.int64)
    pys = np.asarray(inputs["pys"]).astype(np.int64)
    deltaxy = (pxs - pys).astype(np.float32).reshape(-1, 2) + np.float32(H - 1)
    return (predxy, deltaxy), res


def kernel(**inputs):
    outs, _ = _run(inputs, trace=False)
    return outs


# revision 31
# speedup vs baseline: 7.0469x; 1.9302x over previous
"""Trainium2 Bass kernel for DenseRelativeLoc.

Strategy (data-parallel over batch, 8 batches/core x 8 cores):
  - x is host-repacked into a bf16 hi/lo-split gather table
    [128, nb, HW, 12]: partition p, position pos, word j=2*ck+t holds
    bf16 chunk ck (channel ck*128+p) of x (t=0) or of the bf16
    residual (t=1). Streamed to SBUF one batch per DMA (~38KB/part).
  - ONE GPSIMD ap_gather per batch with d=12 fetches, for each of the
    512 sample positions, all 6 channel-chunks x (hi,lo) in a single
    index (ap_gather cost is ~28ns/index independent of d, so fusing
    the chunk dimension into d is ~6x cheaper than per-chunk gathers).
  - 3-layer MLP on TensorE: layer 1 in bf16 with 3-term compensation
    (XhiWhi + XhiWlo + XloWhi ~ fp32 accuracy), layers 2/3 in fp32.
    Activations stay [feature-part, sample-free]; ReLU+bias on DVE.
  - deltaxy is exact integer arithmetic, computed on host.
"""

import sys
import types
import contextlib
import ctypes

sys.path.insert(0, "/opt/trn_rl_repo")

import numpy as np

# ---------------------------------------------------------------- constants
B, C, H, W = 64, 768, 56, 56
HW = H * W            # 3136
S = 256               # points per batch (per side)
NIDX = 2 * S          # 512 gathered positions per batch (px then py)
NH = 256              # hidden width
OUT = 2
NCORES = 8
NB = B // NCORES      # batches per core = 8
KC = C // 128         # channel chunks = 6
D = KC                # gather payload words per index (6 bf16-hi chunks)

# packed f32 const tensor column offsets (per partition)
O_W2 = 0                       # [128, 2, 256] -> 512
O_W3 = O_W2 + 2 * NH           # [128, 2, 2] -> 4
O_B1 = O_W3 + 2 * OUT          # [128, 2]
O_B2 = O_B1 + 2
O_B3 = O_B2 + 2
NCONST = O_B3 + OUT

_PROGRAMS = {}


def _install_ntff_hook():
    """Recreate antenv.axon_hooks (absent in this image) so that
    run_bass_kernel_spmd(trace=True) can register NTFF profiling."""
    import antenv

    if "antenv.axon_hooks" in sys.modules:
        return
    mod = types.ModuleType("antenv.axon_hooks")
    holder = {"hook": None}
    mod.set_axon_ntff_profile_hook = lambda h: holder.__setitem__("hook", h)
    mod.get_axon_ntff_profile_hook = lambda: holder["hook"]
    sys.modules["antenv.axon_hooks"] = mod
    antenv.axon_hooks = mod

    try:
        lib = ctypes.CDLL("/opt/axon/libaxon_pjrt.so")
    except OSError:
        return
    if not hasattr(lib, "axon_start_nrt_profile"):
        return
    lib.axon_start_nrt_profile.argtypes = [ctypes.POINTER(ctypes.c_int64), ctypes.c_size_t]
    lib.axon_start_nrt_profile.restype = ctypes.c_int64
    lib.axon_stop_nrt_profile.argtypes = [ctypes.c_char_p]
    lib.axon_stop_nrt_profile.restype = ctypes.c_int64

    @contextlib.contextmanager
    def _hook(output_dir, device_ids):
        import jax

        jax.devices()
        if device_ids:
            ids = (ctypes.c_int64 * len(device_ids))(*device_ids)
            rc = lib.axon_start_nrt_profile(ids, len(device_ids))
        else:
            rc = lib.axon_start_nrt_profile(None, 0)
        if rc != 0:
            raise RuntimeError(f"axon_start_nrt_profile rc={rc}")
        try:
            yield
        finally:
            n = lib.axon_stop_nrt_profile(str(output_dir).encode())
            print(f"profile: {n} file(s) written to {output_dir}", file=sys.stderr)

    mod.set_axon_ntff_profile_hook(_hook)


def build_program(nb=NB):
    """Build + compile the per-core Bass/Tile program (cached)."""
    if nb in _PROGRAMS:
        return _PROGRAMS[nb]

    import concourse.mybir as mybir
    import concourse.tile as tile
    from concourse import bacc
    from concourse.bass import ts, ds

    f32 = mybir.dt.float32
    bf16 = mybir.dt.bfloat16
    i16 = mybir.dt.int16
    ADD = mybir.AluOpType.add
    MAX = mybir.AluOpType.max

    nc = bacc.Bacc("TRN2", target_bir_lowering=False, debug=False, num_devices=NCORES)

    x_d = nc.dram_tensor("x", [nb * HW, C], bf16, kind="ExternalInput")
    idx_d = nc.dram_tensor("idx", [128, nb, 2, S // 16], i16, kind="ExternalInput")
    wb_d = nc.dram_tensor("wb", [128, 2 * 2 * KC, NH], bf16, kind="ExternalInput")
    cst_d = nc.dram_tensor("cst", [128, NCONST], f32, kind="ExternalInput")
    pred_d = nc.dram_tensor("pred", [128, nb, 2, OUT], f32, kind="ExternalOutput")

    with tile.TileContext(nc) as tc:
        with (
            tc.tile_pool(name="gp", bufs=4) as gp,
            tc.tile_pool(name="wp", bufs=1) as wp,
            tc.tile_pool(name="hp", bufs=2) as hp,
            tc.tile_pool(name="op", bufs=1) as op,
            tc.tile_pool(name="ps1a", bufs=2, space="PSUM") as ps1a,
            tc.tile_pool(name="ps1b", bufs=2, space="PSUM") as ps1b,
            tc.tile_pool(name="ps2", bufs=2, space="PSUM") as ps2,
            tc.tile_pool(name="ps3", bufs=2, space="PSUM") as ps3,
        ):
            cst = wp.tile([128, NCONST], f32, tag="cst")
            wb = wp.tile([128, 2 * 2 * KC, NH], bf16, tag="wb")
            idxt = wp.tile([128, nb, 2, S // 16], i16, tag="idx")
            predt = op.tile([128, nb, 2, OUT], f32, tag="pred")

            nc.sync.dma_start(cst[:], cst_d.ap())
            nc.sync.dma_start(wb[:], wb_d.ap())
            nc.scalar.dma_start(idxt[:], idx_d.ap())

            def w1ap(t, j, nh):  # lhsT [128c, 128n]: t=0 hi, 1 lo; j=xy*KC+ck
                return wb[:, t * 2 * KC + j, ts(nh, 128)]

            def w2ap(nk, mh):
                return cst[:, ds(O_W2 + nk * NH + mh * 128, 128)]

            def w3ap(mk):
                return cst[:, ds(O_W3 + mk * OUT, OUT)]

            for b in range(nb):
                # transpose-mode SDMA gather, one 256-row call per point
                # set (px / py): rows land as gs[xy][p, ck, i] = bf16-hi
                # channel ck*128+p of sample i. 256 idx/call keeps the
                # SWDGE ring occupancy at 98/128 entries.
                gx = gp.tile([128, KC, S], bf16, tag="gx")
                gy = gp.tile([128, KC, S], bf16, tag="gy")
                gs = [gx, gy]
                for xy in range(2):
                    nc.gpsimd.dma_gather(
                        gs[xy][:], x_d.ap(), idxt[:, b, xy, :],
                        num_idxs=S, num_idxs_reg=S,
                        elem_size=C, transpose=True,
                    )

                # ---- layer 1: 2-term bf16 compensation (Xhi(Whi+Wlo))
                h1pa = ps1a.tile([128, NH], f32, tag="h1pa")
                h1pb = ps1b.tile([128, NH], f32, tag="h1pb")
                h1p = [h1pa, h1pb]
                for nh in range(2):
                    first = True
                    for ck in range(KC):
                        for xy in range(2):
                            rhs_hi = gs[xy][:, ck, :]
                            terms = [
                                (w1ap(0, xy * KC + ck, nh), rhs_hi),
                                (w1ap(1, xy * KC + ck, nh), rhs_hi),
                            ]
                            for ti, (wa, ra) in enumerate(terms):
                                last = ck == KC - 1 and xy == 1 and ti == 1
                                nc.tensor.matmul(
                                    h1p[nh][:], wa, ra,
                                    start=first, stop=last,
                                )
                                first = False
                h1 = hp.tile([128, 2, NH], f32, tag="h1")
                for nh in range(2):
                    nc.vector.tensor_scalar(
                        h1[:, nh, :], h1p[nh][:],
                        cst[:, ds(O_B1 + nh, 1)], 0.0, op0=ADD, op1=MAX,
                    )

                # ---- layer 2 (fp32)
                h2p = ps2.tile([128, 2, NH], f32, tag="h2p")
                for mh in range(2):
                    for nk in range(2):
                        nc.tensor.matmul(
                            h2p[:, mh, :],
                            w2ap(nk, mh),
                            h1[:, nk, :],
                            start=(nk == 0),
                            stop=(nk == 1),
                        )
                h2 = hp.tile([128, 2, NH], f32, tag="h2")
                for mh in range(2):
                    nc.vector.tensor_scalar(
                        h2[:, mh, :], h2p[:, mh, :],
                        cst[:, ds(O_B2 + mh, 1)], 0.0, op0=ADD, op1=MAX,
                    )

                # ---- layer 3 (fp32) + b3
                pp = ps3.tile([128, 2, OUT], f32, tag="pp")
                for sh in range(2):
                    for mk in range(2):
                        nc.tensor.matmul(
                            pp[:, sh, :],
                            h2[:, mk, ts(sh, 128)],
                            w3ap(mk),
                            start=(mk == 0),
                            stop=(mk == 1),
                        )
                for sh in range(2):
                    nc.vector.tensor_scalar(
                        predt[:, b, sh, :], pp[:, sh, :],
                        cst[:, ds(O_B3, 1)], None, op0=ADD,
                    )

            nc.sync.dma_start(pred_d.ap(), predt[:])

    nc.compile()
    from concourse.bass_interp import get_hw_module

    nc.m = get_hw_module(nc.m)
    _PROGRAMS[nb] = nc
    return nc


def _prep_core_inputs(x, pxs, pys, W1, b1, W2, b2, W3, b3, nb=NB):
    """Host-side shard + layout massage. Returns list of in_maps."""
    import ml_dtypes

    bf16 = ml_dtypes.bfloat16
    x = np.asarray(x, dtype=np.float32).reshape(B, C, HW)
    pxs = np.asarray(pxs).astype(np.int64)
    pys = np.asarray(pys).astype(np.int64)

    # gather row table [B, HW, C] bf16 (row = all channels of one position)
    xt = np.ascontiguousarray(x.transpose(0, 2, 1))       # [B, HW, C] f32
    xhi = xt.astype(bf16)

    # global row indices (local_batch*HW + pos) per point set, [B, 2, 256]
    # int16, wrapped [16, 16] and tiled to 128 partitions
    xi = pxs[:, :, 0] * H + pxs[:, :, 1]
    yi = pys[:, :, 0] * H + pys[:, :, 1]
    rows = np.stack([xi, yi], axis=1)                     # [B, 2, 256]
    loc = (np.arange(B) % nb) * HW
    glob = (rows + loc[:, None, None]).astype(np.int16)   # [B, 2, 256]
    wrapped = glob.reshape(B, 2, S // 16, 16).transpose(0, 1, 3, 2)  # [B,2,16,16]
    idx128 = np.tile(wrapped, (1, 1, 8, 1))               # [B, 2, 128, 16]

    # weights: hi/lo bf16 W1 chunks as lhsT [128c, j, n]
    W1 = np.asarray(W1, dtype=np.float32)
    w1hi = W1.astype(bf16)
    w1lo = (W1 - w1hi.astype(np.float32)).astype(bf16)
    wk = [m.reshape(2 * KC, 128, NH).transpose(1, 0, 2) for m in (w1hi, w1lo)]
    wb = np.ascontiguousarray(np.concatenate(wk, axis=1))           # [128, 24, 256]

    w2k = np.asarray(W2, dtype=np.float32).reshape(2, 128, NH).transpose(1, 0, 2)
    w3k = np.asarray(W3, dtype=np.float32).reshape(2, 128, OUT).transpose(1, 0, 2)
    b1t = np.asarray(b1, dtype=np.float32).reshape(2, 128).T
    b2t = np.asarray(b2, dtype=np.float32).reshape(2, 128).T
    b3t = np.broadcast_to(np.asarray(b3, dtype=np.float32), (128, OUT))
    cst = np.ascontiguousarray(
        np.concatenate(
            [w2k.reshape(128, -1), w3k.reshape(128, -1), b1t, b2t, b3t], axis=1
        ).astype(np.float32)
    )
    assert cst.shape == (128, NCONST)

    in_maps = []
    for c in range(NCORES):
        sl = slice(c * NB, c * NB + nb)
        in_maps.append(
            {
                "x": np.ascontiguousarray(xhi[sl].reshape(nb * HW, C)),
                "idx": np.ascontiguousarray(idx128[sl].transpose(2, 0, 1, 3)),
                "wb": wb,
                "cst": cst,
            }
        )
    return in_maps


def _assemble_pred(results):
    """Per-core pred [128, NB, 2, 2] -> full predxy [B*S, 2]."""
    parts = []
    for c in range(NCORES):
        p = results[c]["pred"]  # [128, NB, 2, OUT]
        parts.append(np.ascontiguousarray(p.transpose(1, 2, 0, 3)).reshape(NB * 2 * 128, OUT))
    return np.concatenate(parts, axis=0)


def _run(inputs, trace=False):
    _install_ntff_hook()
    from concourse import bass_utils

    nc = build_program()
    in_maps = _prep_core_inputs(**inputs)
    res = bass_utils.run_bass_kernel_spmd(
        nc, in_maps, core_ids=list(range(NCORES)), trace=trace
    )
    predxy = _assemble_pred(res.results)

    pxs = np.asarray(inputs["pxs"]).astype(np.int64)
    pys = np.asarray(inputs["pys"]).astype(np.int64)
    deltaxy = (pxs - pys).astype(np.float32).reshape(-1, 2) + np.float32(H - 1)
    return (predxy, deltaxy), res


def kernel(**inputs):
    outs, _ = _run(inputs, trace=False)
    return outs
